# revision 27
# baseline (speedup 1.0000x reference)
"""MoE routing kernel for Trainium2 (8 NeuronCores, expert-parallel).

Problem (hardcoded shapes): B=4, S=2048, H=1024, I=4096, E=8, capacity=1024.

Mathematical simplification of the reference: softmax routing weights are
strictly positive, so the routing mask is all-ones and the stable argsort of
the (constant) mask is the identity permutation.  Consequently every expert
processes exactly tokens 0..1023 of the flattened [8192, 1024] input, and the
output is nonzero only for those tokens:

    out[n] = sum_e softmax(x[n] @ Wr.T + b)[e] * (relu(x[n] @ Wi[e]) @ Wo[e])

Sharding: expert-parallel.  Each of the 8 cores receives the same 1024-token
slice (pre-transposed to X^T, bf16) and the weights of ONE expert; it
computes that expert's weighted output transposed, [1024 H, 1024 tok] bf16.
The host sums the 8 partial outputs in f32 (the MoE combine), transposes
once, and scatters into the full [4, 2048, 1024] zero tensor.

Per-core device computation (v4, all-bf16):
  startup:  X^T streams in k-major chunks split across the SP queue and the
            otherwise-idle GPSIMD SWDGE queue while the wi tiles stream on
            the ACT queue, so the first layer-1 matmul issues ~2 us in (was
            ~15 us waiting for a monolithic 4 MB f32 X^T upload); a few PE
            warmup matmuls release the HAM clock gate meanwhile.
  layer 1:  inter^T[I, tok] = relu(Wi^T X^T)   (bf16 matmuls, bf16 store)
  router:   interleaved mid-layer-1 (runs at warm PE clock): logits^T[E,tok]
            = Wr_perm X^T (PE, 16 matmuls); exp with the bias folded into
            the ACT activation's per-partition bias operand; sum over
            experts + broadcast to 128 partitions via ones-matmuls.
  layer 2:  outT[H, tok] = Wo^T inter^T, processed per 512-token half so
            each half's epilogue (DVE scale by routing weight + DMA out)
            overlaps the other half's matmuls; output stored bf16.

All weight tensors are pre-packed on host so every DMA is a few contiguous
>=256 KB transfers.  Total per-core DMA ~23 MB, fully hidden under ~225 us
of PE work.
"""

import numpy as np

_CACHE = {}

B, S, H, I, E = 4, 2048, 1024, 4096, 8
CAP = 1024  # capacity = ceil(B*S/E)
N_CORES = 8
KT = H // 128   # 8 k-tiles (H on partitions)
IT = I // 128   # 32 I-tiles
HT = H // 128   # 8 output H-tiles

HALVES = ((0, 512), (512, 1024))


def _build(reps=1):
    import concourse.bacc as bacc
    import concourse.mybir as mybir
    import concourse.tile as tile

    f32 = mybir.dt.float32
    f32r = mybir.dt.float32r
    bf16 = mybir.dt.bfloat16
    AF = mybir.ActivationFunctionType

    nc = bacc.Bacc("TRN2", target_bir_lowering=False, debug=False)

    xtb_d = nc.dram_tensor("xtb", [128, KT, CAP], bf16, kind="ExternalInput")
    rwt_d = nc.dram_tensor("rwt", [128, KT, E], bf16, kind="ExternalInput")
    rb_d = nc.dram_tensor("rb", [E, 1], f32, kind="ExternalInput")
    wi_d = nc.dram_tensor("wi", [IT, 128, KT, 128], bf16, kind="ExternalInput")
    wo_d = nc.dram_tensor("wo", [HT, 128, IT, 128], bf16, kind="ExternalInput")
    outT_d = nc.dram_tensor("outT", [H, CAP], bf16, kind="ExternalOutput")

    with tile.TileContext(nc) as tc:
        with (
            tc.tile_pool(name="const", bufs=1) as const_pool,
            tc.tile_pool(name="wi", bufs=6) as wi_pool,
            tc.tile_pool(name="wo", bufs=HT) as wo_pool,
            tc.tile_pool(name="inter", bufs=1) as inter_pool,
            tc.tile_pool(name="outs", bufs=2) as outs_pool,
            tc.tile_pool(name="small", bufs=2) as small_pool,
            tc.tile_pool(name="psA", bufs=3, space="PSUM") as psA,
            tc.tile_pool(name="psB", bufs=2, space="PSUM") as psB,
        ):
            # ---- resident tensors ----
            # SP queue: X^T chunks first, then router weights + bias (only
            # needed at it=4), later the wo prefetches + output stores.
            # wi rides the ACT queue so the two streams progress in parallel.
            # warm_in memset is deliberately the FIRST vector-queue op so the
            # PE warmup matmuls below can issue as early as possible.
            warm_in = const_pool.tile([1, 512], bf16, name="warm_in")
            nc.vector.memset(warm_in[:], 1.0)

            onesf = const_pool.tile([1, 512], f32)
            nc.vector.memset(onesf[:], 1.0)
            ones_row = const_pool.tile([1, 512], f32r)
            nc.vector.tensor_copy(ones_row[:], onesf[:])
            ones8f = const_pool.tile([8, 1], f32)
            nc.vector.memset(ones8f[:], 1.0)
            ones8 = const_pool.tile([8, 1], f32r)
            nc.vector.tensor_copy(ones8[:], ones8f[:])

            # X^T chunks in k-major order (layer 1's k-loop consumes them in
            # order), front chunks small so the first matmuls issue ~2 us in;
            # the back half rides the otherwise-idle GPSIMD SWDGE queue so
            # the whole 2 MB lands in ~5 us.
            xtb_sb = const_pool.tile([128, KT, CAP], bf16)
            nc.sync.dma_start(xtb_sb[:, 0:1, 0:512], xtb_d.ap()[:, 0:1, 0:512])
            nc.sync.dma_start(xtb_sb[:, 0:1, 512:1024], xtb_d.ap()[:, 0:1, 512:1024])
            nc.sync.dma_start(xtb_sb[:, 1:2, :], xtb_d.ap()[:, 1:2, :])
            nc.sync.dma_start(xtb_sb[:, 2:4, :], xtb_d.ap()[:, 2:4, :])
            nc.gpsimd.dma_start(xtb_sb[:, 4:6, :], xtb_d.ap()[:, 4:6, :])
            nc.gpsimd.dma_start(xtb_sb[:, 6:8, :], xtb_d.ap()[:, 6:8, :])
            rwt_sb = const_pool.tile([128, KT, E], bf16)
            nc.sync.dma_start(rwt_sb[:], rwt_d.ap()[:])
            b_sb = const_pool.tile([E, 1], f32)
            nc.sync.dma_start(b_sb[:], rb_d.ap()[:])

            # PE warmup: dummy matmuls while the first wi/xtb DMAs are in
            # flight, so the HAM clock gate releases (1.2->2.4GHz) before real
            # matmuls start and the cold-clock penalty lands on throwaway
            # work.  The PSUM scratch is never read and the bank is
            # overwritten later by a start=True matmul.
            warm_ps = psB.tile([128, 512], f32, name="warm", tag="half")
            for _ in range(3):
                nc.tensor.matmul(
                    warm_ps[:], warm_in[:, 0:128], warm_in[:], start=True, stop=True
                )

            inter_init = inter_pool.tile([128, IT, CAP], bf16, name="inter")

            def emit_body():
                inter = inter_init
                state = {}

                # -- router stage 1: logits^T = Wr_perm X^T (PE); exp+bias --
                def emit_router_logits():
                    lt = psA.tile([128, CAP], f32, tag="big", name="lt")
                    for lo, hi in HALVES:
                        for k in range(KT):
                            nc.tensor.matmul(
                                lt[0:E, lo:hi],
                                rwt_sb[:, k, :],
                                xtb_sb[:, k, lo:hi],
                                start=(k == 0),
                                stop=(k == KT - 1),
                            )
                    # exp(logit + bias) on ACT; bias is per-partition (=per
                    # expert) so it folds into the activation for free.
                    # max-subtraction skipped: |logits| <~ 8.
                    ex_sb = small_pool.tile([8, CAP], f32r, name="ex")
                    for lo, hi in HALVES:
                        nc.scalar.activation(
                            ex_sb[:, lo:hi], lt[0:E, lo:hi], AF.Exp, bias=b_sb[:]
                        )
                    state["ex"] = ex_sb

                def emit_router_sum():
                    # -- router stage 2: sum over experts + reciprocal --
                    ex_sb = state["ex"]
                    sm = psA.tile([128, CAP], f32, tag="big", name="sm")
                    for lo, hi in HALVES:
                        nc.tensor.matmul(sm[0:1, lo:hi], ones8[:], ex_sb[:, lo:hi])
                    rc = small_pool.tile([1, CAP], f32, name="rc")
                    nc.vector.reciprocal(rc[:], sm[0:1, :])
                    w_row = small_pool.tile([1, CAP], f32r, name="w_row")
                    nc.vector.tensor_mul(w_row[:], ex_sb[0:1, :], rc[:])
                    state["w_row"] = w_row

                def emit_router_bcast():
                    # -- router stage 3: broadcast w to 128 partitions --
                    w_row = state["w_row"]
                    wb = psA.tile([128, CAP], f32, tag="big", name="wb")
                    for lo, hi in HALVES:
                        nc.tensor.matmul(
                            wb[:, lo:hi], ones_row[:, 0:128], w_row[:, lo:hi]
                        )
                    wb_sb = const_pool.tile([128, CAP], f32, name="wb_sb")
                    nc.vector.tensor_copy(wb_sb[:], wb[:])
                    state["wb"] = wb_sb

                # layer-2 weight slabs prefetched on the SP queue mid-layer-1
                # (it is idle after the X^T chunks; upfront they would contend
                # with the startup xtb/wi loads)
                wo_tiles = {}

                def prefetch_wo(ht):
                    wo_tiles[ht] = wo_pool.tile(
                        [128, IT, 128], bf16, name=f"wo_{ht}", tag="wo"
                    )
                    nc.sync.dma_start(wo_tiles[ht][:], wo_d.ap()[ht])

                # -- layer 1 prologue: its 0..2 as ONE k-outer group so the
                # PE consumes each incoming X^T chunk 3x slower than the DMA
                # delivers it (1.28 us vs ~0.8 us per chunk) and runs
                # continuously from first data instead of stalling per-it --
                G = 3
                wi_ts = []
                for g in range(G):
                    wi_t = wi_pool.tile([128, KT, 128], bf16)
                    # k-split so the k=0/1 slices of all three tiles land
                    # before the first k-group needs them
                    nc.scalar.dma_start(wi_t[:, 0:2, :], wi_d.ap()[g][:, 0:2, :])
                    wi_ts.append(wi_t)
                for g in range(G):
                    nc.scalar.dma_start(wi_ts[g][:, 2:, :], wi_d.ap()[g][:, 2:, :])
                p1s = [
                    psA.tile([128, CAP], f32, tag="big", name=f"p1g{g}")
                    for g in range(G)
                ]
                for k in range(KT):
                    for g in range(G):
                        for lo, hi in HALVES:
                            nc.tensor.matmul(
                                p1s[g][:, lo:hi],
                                wi_ts[g][:, k, :],
                                xtb_sb[:, k, lo:hi],
                                start=(k == 0),
                                stop=(k == KT - 1),
                            )
                for g in range(G):
                    nc.scalar.activation(inter[:, g, :], p1s[g][:], AF.Relu)

                # -- layer 1 main loop (with router stages interleaved) --
                for it in range(G, IT):
                    wi_t = wi_pool.tile([128, KT, 128], bf16)
                    nc.scalar.dma_start(wi_t[:], wi_d.ap()[it])
                    p1 = psA.tile([128, CAP], f32, tag="big", name="p1")
                    for k in range(KT):
                        for lo, hi in HALVES:
                            nc.tensor.matmul(
                                p1[:, lo:hi],
                                wi_t[:, k, :],
                                xtb_sb[:, k, lo:hi],
                                start=(k == 0),
                                stop=(k == KT - 1),
                            )
                    nc.scalar.activation(inter[:, it, :], p1[:], AF.Relu)
                    if it == 4:
                        emit_router_logits()
                    elif it == 6:
                        emit_router_sum()
                    elif it == 8:
                        emit_router_bcast()
                    if it >= 8 and (it - 8) % 3 == 0:
                        prefetch_wo((it - 8) // 3)

                wb_sb = state["wb"]

                # -- layer 2: outT = Wo^T inter^T, per-half passes so each
                # half's epilogue overlaps the other half's matmuls --
                for ht in range(HT):
                    wo_t = wo_tiles.pop(ht)
                    for lo, hi in HALVES:
                        p2 = psB.tile([128, 512], f32, name="p2", tag="half")
                        for it2 in range(IT):
                            nc.tensor.matmul(
                                p2[:],
                                wo_t[:, it2, :],
                                inter[:, it2, lo:hi],
                                start=(it2 == 0),
                                stop=(it2 == IT - 1),
                            )
                        o = outs_pool.tile([128, 512], bf16, name="o")
                        nc.vector.tensor_mul(o[:], p2[:], wb_sb[:, lo:hi])
                        nc.sync.dma_start(
                            outT_d.ap()[ht * 128 : (ht + 1) * 128, lo:hi], o[:]
                        )

            for _rep in range(reps):
                emit_body()

    nc.compile()
    return nc


def get_nc():
    if "nc" not in _CACHE:
        _CACHE["nc"] = _build()
    return _CACHE["nc"]


def make_in_maps(x, router_w, router_b, experts_inter, experts_out):
    import ml_dtypes

    bf16 = ml_dtypes.bfloat16

    x_flat = np.asarray(x, dtype=np.float32).reshape(-1, H)
    xt = np.ascontiguousarray(x_flat[:CAP].T)  # [H, CAP]
    # pack to [128, KT, CAP]: xt_p[p, k, n] = xt[k*128 + p, n]
    xtb_p = np.ascontiguousarray(
        xt.reshape(KT, 128, CAP).transpose(1, 0, 2)
    ).astype(bf16)

    wi_bf = np.asarray(experts_inter, dtype=np.float32).astype(bf16)  # [E, H, I]
    wo_bf = np.asarray(experts_out, dtype=np.float32).astype(bf16)    # [E, I, H]

    in_maps = []
    for e in range(N_CORES):
        perm = [e] + [j for j in range(E) if j != e]
        rw = np.asarray(router_w, dtype=np.float32)[perm]  # [E, H]
        rb = np.asarray(router_b, dtype=np.float32)[perm]  # [E]
        # rwt_p[p, k, e] = rw.T[k*128 + p, e]
        rwt_p = np.ascontiguousarray(
            rw.T.reshape(KT, 128, E).transpose(1, 0, 2)
        ).astype(bf16)

        # wi_p[it, p, k, i] = wi[k*128+p, it*128+i]
        wi_p = np.ascontiguousarray(
            wi_bf[e].reshape(KT, 128, IT, 128).transpose(2, 1, 0, 3)
        )
        # wo_p[ht, p, it, h] = wo[it*128+p, ht*128+h]
        wo_p = np.ascontiguousarray(
            wo_bf[e].reshape(IT, 128, HT, 128).transpose(2, 1, 0, 3)
        )
        m = {
            "xtb": xtb_p,
            "rwt": rwt_p,
            "rb": np.ascontiguousarray(rb[:, None]),
            "wi": wi_p,
            "wo": wo_p,
        }
        in_maps.append(m)
    return in_maps


def combine(results):
    partial = np.zeros((H, CAP), dtype=np.float32)
    for r in results:
        partial += np.asarray(r["outT"], dtype=np.float32)
    out = np.zeros((B * S, H), dtype=np.float32)
    out[:CAP] = partial.T
    return out.reshape(B, S, H)


def kernel(x, router_w, router_b, experts_inter, experts_out):
    from concourse import bass_utils

    nc = get_nc()
    in_maps = make_in_maps(x, router_w, router_b, experts_inter, experts_out)
    res = bass_utils.run_bass_kernel_spmd(nc, in_maps, core_ids=list(range(N_CORES)))
    return combine(res.results)


# revision 30
# speedup vs baseline: 1.0367x; 1.0367x over previous
"""MoE routing kernel for Trainium2 (8 NeuronCores, expert-parallel).

Problem (hardcoded shapes): B=4, S=2048, H=1024, I=4096, E=8, capacity=1024.

Mathematical simplification of the reference: softmax routing weights are
strictly positive, so the routing mask is all-ones and the stable argsort of
the (constant) mask is the identity permutation.  Consequently every expert
processes exactly tokens 0..1023 of the flattened [8192, 1024] input, and the
output is nonzero only for those tokens:

    out[n] = sum_e softmax(x[n] @ Wr.T + b)[e] * (relu(x[n] @ Wi[e]) @ Wo[e])

Sharding: expert-parallel.  Each of the 8 cores receives the same 1024-token
slice (pre-transposed to X^T, bf16) and the weights of ONE expert; it
computes that expert's weighted output transposed, [1024 H, 1024 tok] bf16.
The host sums the 8 partial outputs in f32 (the MoE combine), transposes
once, and scatters into the full [4, 2048, 1024] zero tensor.

Per-core device computation (v4, all-bf16):
  startup:  X^T streams in k-major chunks split across the SP queue and the
            otherwise-idle GPSIMD SWDGE queue while the wi tiles stream on
            the ACT queue, so the first layer-1 matmul issues ~2 us in (was
            ~15 us waiting for a monolithic 4 MB f32 X^T upload); a few PE
            warmup matmuls release the HAM clock gate meanwhile.
  layer 1:  inter^T[I, tok] = relu(Wi^T X^T)   (bf16 matmuls, bf16 store)
  router:   interleaved mid-layer-1 (runs at warm PE clock): logits^T[E,tok]
            = Wr_perm X^T (PE, 16 matmuls); exp with the bias folded into
            the ACT activation's per-partition bias operand; sum over
            experts + broadcast to 128 partitions via ones-matmuls.
  layer 2:  outT[H, tok] = Wo^T inter^T, processed per 512-token half so
            each half's epilogue (DVE scale by routing weight + DMA out)
            overlaps the other half's matmuls; output stored bf16.

All weight tensors are pre-packed on host so every DMA is a few contiguous
>=256 KB transfers.  Total per-core DMA ~23 MB, fully hidden under ~225 us
of PE work.
"""

import numpy as np

_CACHE = {}

B, S, H, I, E = 4, 2048, 1024, 4096, 8
CAP = 1024  # capacity = ceil(B*S/E)
N_CORES = 8
KT = H // 128   # 8 k-tiles (H on partitions)
IT = I // 128   # 32 I-tiles
HT = H // 128   # 8 output H-tiles

HALVES = ((0, 512), (512, 1024))


def _build(reps=1):
    import concourse.bacc as bacc
    import concourse.mybir as mybir
    import concourse.tile as tile

    f32 = mybir.dt.float32
    f32r = mybir.dt.float32r
    bf16 = mybir.dt.bfloat16
    AF = mybir.ActivationFunctionType

    nc = bacc.Bacc("TRN2", target_bir_lowering=False, debug=False)

    xtb_d = nc.dram_tensor("xtb", [128, KT, CAP], bf16, kind="ExternalInput")
    rwt_d = nc.dram_tensor("rwt", [128, KT, E], bf16, kind="ExternalInput")
    rb_d = nc.dram_tensor("rb", [E, 1], f32, kind="ExternalInput")
    wi_d = nc.dram_tensor("wi", [IT, 128, KT, 128], bf16, kind="ExternalInput")
    wo_d = nc.dram_tensor("wo", [HT, 128, IT, 128], bf16, kind="ExternalInput")
    outT_d = nc.dram_tensor("outT", [H, CAP], bf16, kind="ExternalOutput")

    with tile.TileContext(nc) as tc:
        with (
            tc.tile_pool(name="const", bufs=1) as const_pool,
            tc.tile_pool(name="wi", bufs=8) as wi_pool,
            tc.tile_pool(name="wo", bufs=HT) as wo_pool,
            tc.tile_pool(name="inter", bufs=1) as inter_pool,
            tc.tile_pool(name="outs", bufs=2) as outs_pool,
            tc.tile_pool(name="small", bufs=2) as small_pool,
            tc.tile_pool(name="psA", bufs=3, space="PSUM") as psA,
            tc.tile_pool(name="psB", bufs=2, space="PSUM") as psB,
        ):
            # ---- resident tensors ----
            # SP queue: X^T chunks first, then router weights + bias (only
            # needed at it=4), later the wo prefetches + output stores.
            # wi rides the ACT queue so the two streams progress in parallel.
            # warm_in memset is deliberately the FIRST vector-queue op so the
            # PE warmup matmuls below can issue as early as possible.
            warm_in = const_pool.tile([1, 512], bf16, name="warm_in")
            nc.vector.memset(warm_in[:], 1.0)

            onesf = const_pool.tile([1, 512], f32)
            nc.vector.memset(onesf[:], 1.0)
            ones_row = const_pool.tile([1, 512], f32r)
            nc.vector.tensor_copy(ones_row[:], onesf[:])
            ones8f = const_pool.tile([8, 1], f32)
            nc.vector.memset(ones8f[:], 1.0)
            ones8 = const_pool.tile([8, 1], f32r)
            nc.vector.tensor_copy(ones8[:], ones8f[:])

            # X^T chunks in k-major order (layer 1's k-loop consumes them in
            # order), front chunks small so the first matmuls issue ~2 us in;
            # the back half rides the otherwise-idle GPSIMD SWDGE queue so
            # the whole 2 MB lands in ~5 us.
            xtb_sb = const_pool.tile([128, KT, CAP], bf16)
            nc.sync.dma_start(xtb_sb[:, 0:1, 0:512], xtb_d.ap()[:, 0:1, 0:512])
            nc.sync.dma_start(xtb_sb[:, 0:1, 512:1024], xtb_d.ap()[:, 0:1, 512:1024])
            nc.sync.dma_start(xtb_sb[:, 1:2, :], xtb_d.ap()[:, 1:2, :])
            nc.sync.dma_start(xtb_sb[:, 2:4, :], xtb_d.ap()[:, 2:4, :])
            nc.gpsimd.dma_start(xtb_sb[:, 4:6, :], xtb_d.ap()[:, 4:6, :])
            nc.gpsimd.dma_start(xtb_sb[:, 6:8, :], xtb_d.ap()[:, 6:8, :])
            rwt_sb = const_pool.tile([128, KT, E], bf16)
            nc.sync.dma_start(rwt_sb[:], rwt_d.ap()[:])
            b_sb = const_pool.tile([E, 1], f32)
            nc.sync.dma_start(b_sb[:], rb_d.ap()[:])

            # PE warmup: dummy matmuls while the first wi/xtb DMAs are in
            # flight, so the HAM clock gate releases (1.2->2.4GHz) before real
            # matmuls start and the cold-clock penalty lands on throwaway
            # work.  The PSUM scratch is never read and the bank is
            # overwritten later by a start=True matmul.
            warm_ps = psB.tile([128, 512], f32, name="warm", tag="half")
            for _ in range(3):
                nc.tensor.matmul(
                    warm_ps[:], warm_in[:, 0:128], warm_in[:], start=True, stop=True
                )

            inter_init = inter_pool.tile([128, IT, CAP], bf16, name="inter")

            def emit_body():
                inter = inter_init
                state = {}

                # -- router stage 1: logits^T = Wr_perm X^T (PE); exp+bias --
                def emit_router_logits():
                    lt = psA.tile([128, CAP], f32, tag="big", name="lt")
                    for lo, hi in HALVES:
                        for k in range(KT):
                            nc.tensor.matmul(
                                lt[0:E, lo:hi],
                                rwt_sb[:, k, :],
                                xtb_sb[:, k, lo:hi],
                                start=(k == 0),
                                stop=(k == KT - 1),
                            )
                    # exp(logit + bias) on ACT; bias is per-partition (=per
                    # expert) so it folds into the activation for free.
                    # max-subtraction skipped: |logits| <~ 8.
                    ex_sb = small_pool.tile([8, CAP], f32r, name="ex")
                    for lo, hi in HALVES:
                        nc.scalar.activation(
                            ex_sb[:, lo:hi], lt[0:E, lo:hi], AF.Exp, bias=b_sb[:]
                        )
                    state["ex"] = ex_sb

                def emit_router_sum():
                    # -- router stage 2: sum over experts + reciprocal --
                    ex_sb = state["ex"]
                    sm = psA.tile([128, CAP], f32, tag="big", name="sm")
                    for lo, hi in HALVES:
                        nc.tensor.matmul(sm[0:1, lo:hi], ones8[:], ex_sb[:, lo:hi])
                    rc = small_pool.tile([1, CAP], f32, name="rc")
                    nc.vector.reciprocal(rc[:], sm[0:1, :])
                    w_row = small_pool.tile([1, CAP], f32r, name="w_row")
                    nc.vector.tensor_mul(w_row[:], ex_sb[0:1, :], rc[:])
                    state["w_row"] = w_row

                def emit_router_bcast():
                    # -- router stage 3: broadcast w to 128 partitions --
                    w_row = state["w_row"]
                    wb = psA.tile([128, CAP], f32, tag="big", name="wb")
                    for lo, hi in HALVES:
                        nc.tensor.matmul(
                            wb[:, lo:hi], ones_row[:, 0:128], w_row[:, lo:hi]
                        )
                    wb_sb = const_pool.tile([128, CAP], f32, name="wb_sb")
                    nc.vector.tensor_copy(wb_sb[:], wb[:])
                    state["wb"] = wb_sb

                # layer-2 weight slabs prefetched on the SP queue mid-layer-1
                # (it is idle after the X^T chunks; upfront they would contend
                # with the startup xtb/wi loads)
                wo_tiles = {}

                def prefetch_wo(ht):
                    wo_tiles[ht] = wo_pool.tile(
                        [128, IT, 128], bf16, name=f"wo_{ht}", tag="wo"
                    )
                    nc.sync.dma_start(wo_tiles[ht][:], wo_d.ap()[ht])

                # -- layer 1 prologue: its 0..2 as ONE k-outer group so the
                # PE consumes each incoming X^T chunk 3x slower than the DMA
                # delivers it (1.28 us vs ~0.8 us per chunk) and runs
                # continuously from first data instead of stalling per-it --
                G = 3
                wi_ts = []
                for g in range(G):
                    wi_t = wi_pool.tile([128, KT, 128], bf16)
                    # k-split so the k=0/1 slices of all three tiles land
                    # before the first k-group needs them
                    nc.scalar.dma_start(wi_t[:, 0:2, :], wi_d.ap()[g][:, 0:2, :])
                    wi_ts.append(wi_t)
                for g in range(G):
                    nc.scalar.dma_start(wi_ts[g][:, 2:, :], wi_d.ap()[g][:, 2:, :])
                p1s = [
                    psA.tile([128, CAP], f32, tag="big", name=f"p1g{g}")
                    for g in range(G)
                ]
                for k in range(KT):
                    for g in range(G):
                        for lo, hi in HALVES:
                            nc.tensor.matmul(
                                p1s[g][:, lo:hi],
                                wi_ts[g][:, k, :],
                                xtb_sb[:, k, lo:hi],
                                start=(k == 0),
                                stop=(k == KT - 1),
                            )
                for g in range(G):
                    nc.scalar.activation(inter[:, g, :], p1s[g][:], AF.Relu)

                # -- layer 1 main loop (with router stages interleaved) --
                for it in range(G, IT):
                    wi_t = wi_pool.tile([128, KT, 128], bf16)
                    nc.scalar.dma_start(wi_t[:], wi_d.ap()[it])
                    p1 = psA.tile([128, CAP], f32, tag="big", name="p1")
                    for k in range(KT):
                        for lo, hi in HALVES:
                            nc.tensor.matmul(
                                p1[:, lo:hi],
                                wi_t[:, k, :],
                                xtb_sb[:, k, lo:hi],
                                start=(k == 0),
                                stop=(k == KT - 1),
                            )
                    nc.scalar.activation(inter[:, it, :], p1[:], AF.Relu)
                    if it == 4:
                        emit_router_logits()
                    elif it == 6:
                        emit_router_sum()
                    elif it == 8:
                        emit_router_bcast()
                    if it >= 8 and (it - 8) % 3 == 0:
                        prefetch_wo((it - 8) // 3)

                wb_sb = state["wb"]

                # -- layer 2: outT = Wo^T inter^T, per-half passes so each
                # half's epilogue overlaps the other half's matmuls --
                for ht in range(HT):
                    wo_t = wo_tiles.pop(ht)
                    for lo, hi in HALVES:
                        p2 = psB.tile([128, 512], f32, name="p2", tag="half")
                        for it2 in range(IT):
                            nc.tensor.matmul(
                                p2[:],
                                wo_t[:, it2, :],
                                inter[:, it2, lo:hi],
                                start=(it2 == 0),
                                stop=(it2 == IT - 1),
                            )
                        o = outs_pool.tile([128, 512], bf16, name="o")
                        nc.vector.tensor_mul(o[:], p2[:], wb_sb[:, lo:hi])
                        nc.sync.dma_start(
                            outT_d.ap()[ht * 128 : (ht + 1) * 128, lo:hi], o[:]
                        )

            for _rep in range(reps):
                emit_body()

    nc.compile()
    return nc


def get_nc():
    if "nc" not in _CACHE:
        _CACHE["nc"] = _build()
    return _CACHE["nc"]


def make_in_maps(x, router_w, router_b, experts_inter, experts_out):
    import ml_dtypes

    bf16 = ml_dtypes.bfloat16

    x_flat = np.asarray(x, dtype=np.float32).reshape(-1, H)
    xt = np.ascontiguousarray(x_flat[:CAP].T)  # [H, CAP]
    # pack to [128, KT, CAP]: xt_p[p, k, n] = xt[k*128 + p, n]
    xtb_p = np.ascontiguousarray(
        xt.reshape(KT, 128, CAP).transpose(1, 0, 2)
    ).astype(bf16)

    wi_bf = np.asarray(experts_inter, dtype=np.float32).astype(bf16)  # [E, H, I]
    wo_bf = np.asarray(experts_out, dtype=np.float32).astype(bf16)    # [E, I, H]

    in_maps = []
    for e in range(N_CORES):
        perm = [e] + [j for j in range(E) if j != e]
        rw = np.asarray(router_w, dtype=np.float32)[perm]  # [E, H]
        rb = np.asarray(router_b, dtype=np.float32)[perm]  # [E]
        # rwt_p[p, k, e] = rw.T[k*128 + p, e]
        rwt_p = np.ascontiguousarray(
            rw.T.reshape(KT, 128, E).transpose(1, 0, 2)
        ).astype(bf16)

        # wi_p[it, p, k, i] = wi[k*128+p, it*128+i]
        wi_p = np.ascontiguousarray(
            wi_bf[e].reshape(KT, 128, IT, 128).transpose(2, 1, 0, 3)
        )
        # wo_p[ht, p, it, h] = wo[it*128+p, ht*128+h]
        wo_p = np.ascontiguousarray(
            wo_bf[e].reshape(IT, 128, HT, 128).transpose(2, 1, 0, 3)
        )
        m = {
            "xtb": xtb_p,
            "rwt": rwt_p,
            "rb": np.ascontiguousarray(rb[:, None]),
            "wi": wi_p,
            "wo": wo_p,
        }
        in_maps.append(m)
    return in_maps


def combine(results):
    partial = np.zeros((H, CAP), dtype=np.float32)
    for r in results:
        partial += np.asarray(r["outT"], dtype=np.float32)
    out = np.zeros((B * S, H), dtype=np.float32)
    out[:CAP] = partial.T
    return out.reshape(B, S, H)


def kernel(x, router_w, router_b, experts_inter, experts_out):
    from concourse import bass_utils

    nc = get_nc()
    in_maps = make_in_maps(x, router_w, router_b, experts_inter, experts_out)
    res = bass_utils.run_bass_kernel_spmd(nc, in_maps, core_ids=list(range(N_CORES)))
    return combine(res.results)
